# revision 46
# baseline (speedup 1.0000x reference)
"""BertSelfAttention (B=2, S=2048, H=1024, 16 heads x 64) on 8 TRN2 NeuronCores.

Sharding: data parallel on batch (4 cores per batch) x tensor parallel on
heads (4 heads per core). No cross-core comms; each core computes
out[b, :, 256*g:256*(g+1)] for its head group g.

v6c: global 128-slot scheduler. The exp stream (ACT engine, the ~146us
floor) is the pacing clock; everything else is placed around it:
  - hiddenT via 16 X-bar transposes ([1024,128] x 2 groups) pipelined
    against the early exp slots; weights land host-pre-arranged so the DMA
    is contiguous.
  - PE warmed up with a few junk matmuls so the HAM clock gate is at 2.4GHz
    when the first projection issues.
  - ctx matmuls lag the exp stream by 16 slots (deep et buffering) so v/qk
    filler projections spread across all 8 sweeps instead of starving the
    ACT engine in sweep 0; the lag tapers at the end for a short tail.
  - pair-1 output chunks DMA out per-qb as soon as both pairs wrote them.

Per-core pipeline:
  A) hiddenT [128(j), 2048(s)] bf16 via X-bar DMA transposes (sync, serial)
  B) kT/qT [128(d of head pair), 2048(s)] bf16 (1/8 scale + bias folded),
     V [128(s), 4heads, 65] bf16 with ones column (denominator for free)
  C) per slot (pair, qb=512, k): scoresT h0|h1 packed -> one exp [128,1024]
     (mask bias) -> bf16 et; lagged ctxT[65, 512] += v_ext.T @ et per head
  D) PE-transpose ctxT (bf16) -> [q, 65], DVE reciprocal + scale, DMA out
"""

import ml_dtypes
import numpy as np

import concourse.bass as bass
import concourse.tile as tile
from concourse import bacc, mybir
from concourse.bass_utils import run_bass_kernel_spmd
from concourse.masks import make_identity

F32 = mybir.dt.float32
BF16 = mybir.dt.bfloat16
EXP = mybir.ActivationFunctionType.Exp

B, S, H = 2, 2048, 1024
NH, HD = 16, 64
NCORES = 8
HPC = 4  # heads per core
DPC = HPC * HD  # 256 output dims per core
SC = S // 128  # 16 s/k chunks
JC = H // 128  # 8 contraction chunks
QB = 512  # q block in attention inner loop
NQB = S // QB  # 4
NSLOT = 2 * NQB * SC  # 128 global (pair, qb, k) slots
LAG = 24  # ctx stream lag behind the exp stream, in slots


def build():
    nc = bacc.Bacc(
        "TRN2",
        target_bir_lowering=False,
        debug=False,
        enable_asserts=False,
        num_devices=NCORES,
    )
    # hidb arrives host-pre-transposed AND pre-tiled as [128, 4, JC, 512]
    # (p, s-quarter, j, s-within) so each s-quarter wave is ONE fully-
    # contiguous DMA (8KB runs per partition on both sides - the DMA
    # engine is packet-rate-bound, so big runs = full bandwidth)
    hidb = nc.dram_tensor("hidb", [128, 4 * JC * 512], BF16, kind="ExternalInput").ap()
    # wq/wk arrive host-pre-arranged as [128, 2(p-half), JC, 128] so each
    # half loads as one 2KB-row contiguous DMA; wv stays [128, JC, DPC]
    wq = nc.dram_tensor("wq", [128, JC * DPC], BF16, kind="ExternalInput").ap()
    wk = nc.dram_tensor("wk", [128, JC * DPC], BF16, kind="ExternalInput").ap()
    wv = nc.dram_tensor("wv", [128, JC * DPC], BF16, kind="ExternalInput").ap()
    bqs = nc.dram_tensor("bqs", [128, 2], F32, kind="ExternalInput").ap()
    bks = nc.dram_tensor("bks", [128, 2], F32, kind="ExternalInput").ap()
    bvs = nc.dram_tensor("bvs", [1, DPC], BF16, kind="ExternalInput").ap()
    mask = nc.dram_tensor("mask", [128, SC], F32, kind="ExternalInput").ap()
    out = nc.dram_tensor("out", [S, DPC], F32, kind="ExternalOutput").ap()

    with tile.TileContext(nc) as tc:
        with (
            tc.tile_pool(name="persist", bufs=1) as persist,
            tc.tile_pool(name="etp", bufs=LAG + 3) as etp,
            tc.tile_pool(name="ctsp", bufs=2) as ctsp,
            tc.tile_pool(name="rcp", bufs=4) as rcp,
            tc.tile_pool(name="scps", bufs=2, space="PSUM") as scps,
            tc.tile_pool(name="ctxps", bufs=1, space="PSUM") as ctxps,
            tc.tile_pool(name="vdps", bufs=2, space="PSUM") as vdps,
        ):
            # DMA schedule: hidb is host-pre-transposed, so hidT loads are
            # plain contiguous DMAs. The critical bytes before the first
            # scores (hidT s0:512 all j + p0-halves of wk/wq + biases/mask)
            # are split across BOTH hwdge queues (sync + Activation) for
            # aggregate HBM bandwidth; everything else streams afterwards on
            # the sync queue ONLY, in deadline order, because DMA issues on
            # the Activation queue would block the exp stream.
            hidTall = persist.tile([128, 4, JC, 512], BF16, tag="hT", name="hT")
            hidb4 = hidb.rearrange("p (h c s) -> p h c s", h=4, s=512)
            # wq/wk tiles are [128, 2(p-half), JC, 128]; wv stays j-major
            w_sb = {}
            for name in ("wk", "wq"):
                w_sb[name] = persist.tile(
                    [128, 2, JC, 128], BF16, tag=name, name=f"w_{name}"
                )
            w_sb["wv"] = persist.tile([128, JC, DPC], BF16, tag="wv", name="w_wv")
            wk4 = wk.rearrange("p (h c n) -> p h c n", h=2, n=128)
            wq4 = wq.rearrange("p (h c n) -> p h c n", h=2, n=128)
            bks_sb = persist.tile([128, 2], F32, tag="bks")
            bqs_sb = persist.tile([128, 2], F32, tag="bqs")
            mask_sb = persist.tile([128, SC], F32, tag="mask")
            bvs_sb = persist.tile([1, DPC], BF16, tag="bvs")

            # DMA completion is latency-bound (~1.5us fixed per DMA, ~4
            # outstanding per queue) and sub-2KB elements get chopped into
            # tiny packets (~30GB/s), so critical transfers are FEW and
            # FULLY CONTIGUOUS. The p0 weight halves (2KB rows) ride the
            # Activation queue (done well before exps; NOTE bulk/3D-strided
            # DMAs there permanently slow every later ACTIVATE ~20% -
            # verified empirically - so only small 2D weights go here);
            # wave A of hid (2MB) is one big DMA on sync.
            nc.scalar.dma_start(w_sb["wk"][:, 0], wk4[:, 0])
            nc.scalar.dma_start(w_sb["wq"][:, 0], wq4[:, 0])
            nc.scalar.dma_start(bks_sb[:], bks)
            nc.scalar.dma_start(bqs_sb[:], bqs)
            nc.scalar.dma_start(mask_sb[:], mask)
            nc.sync.dma_start(hidTall[:, 0], hidb4[:, 0])
            nc.sync.dma_start(hidTall[:, 1], hidb4[:, 1])
            # rest, sync only, in deadline order: wv+bvs (v fills from slot
            # 6), s1024:2048 (g2/g3 scores from slot ~16), p1 weights
            # (fills from slot 42)
            nc.sync.dma_start(
                w_sb["wv"][:], wv.rearrange("p (c n) -> p c n", n=DPC)
            )
            nc.sync.dma_start(bvs_sb[:], bvs)
            nc.sync.dma_start(hidTall[:, 2], hidb4[:, 2])
            nc.sync.dma_start(hidTall[:, 3], hidb4[:, 3])
            nc.sync.dma_start(w_sb["wk"][:, 1], wk4[:, 1])
            nc.sync.dma_start(w_sb["wq"][:, 1], wq4[:, 1])

            ones1_f = persist.tile([1, 128], F32, tag="ones1f")
            nc.vector.memset(ones1_f[:], 1.0)
            # warm the ACT exp table during startup
            warm = persist.tile([1, 1], F32, tag="warm")
            nc.scalar.activation(warm[:], ones1_f[:, 0:1], EXP)

            # PE warmup: real array work (junk accumulating matmuls) so the
            # HAM activity monitor lifts the clock gate before the first
            # projection matmuls issue
            junk = persist.tile([128, 512], BF16, tag="junk")
            nc.vector.memset(junk[:], 0.0)

            def spam(n):
                # junk matmuls rotating through the (still unused) scores
                # PSUM ring so consecutive WAW chains overlap and the array
                # duty cycle stays high enough to hold the HAM gate open
                for i in range(n):
                    sp = scps.tile([8, 512], F32, tag="sc", name="spam_t")
                    nc.tensor.matmul(
                        sp[:], junk[:, 0:8], junk[:], start=True, stop=True
                    )

            # bridge the PE from preamble (~3us) all the way to the wave-A
            # landing (~20us) so the g0 projections run at full clock
            spam(28)

            ident = persist.tile([128, 128], F32, tag="ident")
            make_identity(nc, ident[:])
            ident_bf = persist.tile([65, 65], BF16, tag="identbf")
            nc.vector.tensor_copy(ident_bf[:], ident[0:65, 0:65])
            ones1 = persist.tile([1, 128], BF16, tag="ones1")
            nc.vector.tensor_copy(ones1[:], ones1_f[:])
            ones4_f = persist.tile([128, HPC], F32, tag="ones4f")
            nc.vector.memset(ones4_f[:], 1.0)

            # persistent activations
            qT = [
                persist.tile([128, S], BF16, tag=f"qT{p}", name=f"qT{p}")
                for p in range(2)
            ]
            kT = [
                persist.tile([128, S], BF16, tag=f"kT{p}", name=f"kT{p}")
                for p in range(2)
            ]
            v_sb = [
                persist.tile([128, HPC, 65], BF16, tag=f"v{s}", name=f"v{s}")
                for s in range(SC)
            ]
            out_sb = [
                persist.tile([128, DPC], F32, tag=f"o{s}", name=f"o{s}")
                for s in range(SC)
            ]

            def hid_g(g, j):
                # 512-wide column group g of hiddenT chunk j
                return hidTall[:, g, j, :]

            def hid_s(s, j):
                # 128-wide column chunk s of hiddenT chunk j
                h, ss = divmod(s, 4)
                return hidTall[:, h, j, ss * 128 : (ss + 1) * 128]

            def qk_pieces(wname, dst, bias, p, g):
                # one 512-wide output group (1 PSUM bank) of the qT/kT proj,
                # split into two 4-chunk pieces for fine-grained scheduling
                box = {}

                def piece_a():
                    ps = vdps.tile([128, 512], F32, tag="vd", name="projps_t")
                    box["ps"] = ps
                    for j in range(4):
                        nc.tensor.matmul(
                            ps[:],
                            w_sb[wname][:, p, j, :],
                            hid_g(g, j),
                            start=(j == 0),
                            stop=False,
                        )

                def piece_b():
                    ps = box["ps"]
                    for j in range(4, JC):
                        nc.tensor.matmul(
                            ps[:],
                            w_sb[wname][:, p, j, :],
                            hid_g(g, j),
                            start=False,
                            stop=(j == JC - 1),
                        )
                    nc.vector.tensor_scalar_add(
                        dst[p][:, g * 512 : (g + 1) * 512], ps[:], bias[:, p : p + 1]
                    )

                return piece_a, piece_b

            def v_pieces(s):
                # v projection for s-chunk s, split in two for scheduling
                box = {}

                def piece_a():
                    ps = vdps.tile([128, DPC], F32, tag="vd", name="vps_t")
                    box["ps"] = ps
                    for j in range(4):
                        nc.tensor.matmul(
                            ps[:],
                            hid_s(s, j),
                            w_sb["wv"][:, j, :],
                            start=(j == 0),
                            stop=False,
                        )

                def piece_b():
                    ps = box["ps"]
                    for j in range(4, JC):
                        nc.tensor.matmul(
                            ps[:],
                            hid_s(s, j),
                            w_sb["wv"][:, j, :],
                            start=False,
                            stop=False,
                        )
                    nc.tensor.matmul(
                        ps[:], ones1[:], bvs_sb[:], start=False, stop=True
                    )
                    ps3 = ps.rearrange("p (h c) -> p h c", h=HPC)
                    nc.vector.tensor_copy(v_sb[s][:, :, 0:HD], ps3[:])
                    nc.vector.tensor_copy(
                        v_sb[s][:, :, HD : HD + 1],
                        ones4_f[:].rearrange("p (h o) -> p h o", o=1),
                    )

                return piece_a, piece_b

            # ---- global slot schedule ----
            # pair-0 front: (qb0,k0-7),(qb1,k0-7),(qb0,k8-15),(qb1,k8-15)
            # so the second transpose wave is first needed at slot 16, out of
            # reach of X-bar timing variance; everything after is qb-major
            slots = (
                [(0, 0, k) for k in range(8)]
                + [(0, 1, k) for k in range(8)]
                + [(0, 0, k) for k in range(8, SC)]
                + [(0, 1, k) for k in range(8, SC)]
                + [(0, qb, k) for qb in (2, 3) for k in range(SC)]
                + [(1, qb, k) for qb in range(NQB) for k in range(SC)]
            )
            # ctx emission order: qb-grouped (one live ctx psum pair), each
            # (pair,qb,k)'s et comes from its exp slot
            ctx_order = [
                (pair, qb, k)
                for pair in range(2)
                for qb in range(NQB)
                for k in range(SC)
            ]
            slot_of = {pqk: i for i, pqk in enumerate(slots)}

            # fillers: slot index -> list of closures, placed by deadline:
            #   kT[p] group g needed by scores slot (64p + 4g)
            #   qT[p] group g needed by scores slot (64p + 16g)
            #   v(s) needed by ctx(p0, qb0, s) emitted at slot s + LAG
            # hidT availability: s 0:1024 ~ slot 0, s 1024:2048 ~ slot 8
            fill = {}

            def place(slot_a, slot_b, fns):
                a, b = fns
                fill.setdefault(slot_a, []).append(a)
                fill.setdefault(slot_b, []).append(b)

            place(0, 1, qk_pieces("wk", kT, bks_sb, 0, 1))
            place(5, 6, qk_pieces("wq", qT, bqs_sb, 0, 1))
            place(11, 12, qk_pieces("wk", kT, bks_sb, 0, 2))
            place(17, 18, qk_pieces("wk", kT, bks_sb, 0, 3))
            place(24, 25, qk_pieces("wq", qT, bqs_sb, 0, 2))
            place(40, 41, qk_pieces("wq", qT, bqs_sb, 0, 3))
            place(42, 46, qk_pieces("wk", kT, bks_sb, 1, 0))
            place(50, 54, qk_pieces("wk", kT, bks_sb, 1, 1))
            place(58, 62, qk_pieces("wk", kT, bks_sb, 1, 2))
            place(66, 70, qk_pieces("wk", kT, bks_sb, 1, 3))
            place(43, 47, qk_pieces("wq", qT, bqs_sb, 1, 0))
            place(51, 55, qk_pieces("wq", qT, bqs_sb, 1, 1))
            place(59, 63, qk_pieces("wq", qT, bqs_sb, 1, 2))
            place(80, 81, qk_pieces("wq", qT, bqs_sb, 1, 3))
            # v projections as halves, one half per slot; starts at slot 6
            # (wv lands ~17us; earlier placement would freeze the in-order
            # PE queue on the wv DMA); v(s) must complete by ctx(p0,q0,s) =
            # slot LAG+s
            for s in range(SC):
                place(6 + 2 * s, 7 + 2 * s, v_pieces(s))

            # ctx emission quota per slot: nothing for the first LAG slots,
            # 1 steady, with a gentle catch-up (one extra on odd slots 45..79
            # and a few more mid-kernel) so the lag shrinks to ~2 well before
            # the end and no slot carries a double-ctx burst late
            ctx_quota = [0] * NSLOT
            for i in range(LAG, NSLOT):
                ctx_quota[i] = 1
            for i in (44, 48, 52, 56, 60, 64, 68, 72, 74, 76, 78,
                      82, 84, 88, 90, 94, 96, 100, 102, 106, 108, 110):
                ctx_quota[i] += 1
            assert sum(ctx_quota) == NSLOT - 2, sum(ctx_quota)

            # critical-path g0 projections: all inputs arrive together (wave
            # A + p0 weights), k first so its bias add overlaps the q MMs
            kps0 = vdps.tile([128, 512], F32, tag="vd", name="kps0")
            qps0 = vdps.tile([128, 512], F32, tag="vd", name="qps0")
            for j in range(JC):
                nc.tensor.matmul(
                    kps0[:], w_sb["wk"][:, 0, j, :], hid_g(0, j),
                    start=(j == 0), stop=(j == JC - 1),
                )
            nc.vector.tensor_scalar_add(kT[0][:, 0:512], kps0[:], bks_sb[:, 0:1])
            for j in range(JC):
                nc.tensor.matmul(
                    qps0[:], w_sb["wq"][:, 0, j, :], hid_g(0, j),
                    start=(j == 0), stop=(j == JC - 1),
                )
            nc.vector.tensor_scalar_add(qT[0][:, 0:512], qps0[:], bqs_sb[:, 0:1])

            sts = {}
            postq = []  # deferred postlude chunk units, drained 1/slot

            def emit_scores(i):
                pair, qb, k = slots[i]
                st = scps.tile([128, 2 * QB], F32, tag="sc", name="sc_t")
                qs = qb * QB
                # adjacent emission, opposite row groups -> the PE runs
                # these two K=64 scores matmuls concurrently
                nc.tensor.matmul(
                    st[:, 0:QB],
                    kT[pair][0:64, k * 128 : (k + 1) * 128],
                    qT[pair][0:64, qs : qs + QB],
                    start=True,
                    stop=True,
                )
                nc.tensor.matmul(
                    st[:, QB : 2 * QB],
                    kT[pair][64:128, k * 128 : (k + 1) * 128],
                    qT[pair][64:128, qs : qs + QB],
                    start=True,
                    stop=True,
                )
                sts[i] = st

            ets = {}
            ctx_tiles = {}
            cptr = 0  # next ctx slot-index to emit

            def emit_ctx():
                nonlocal cptr
                c = cptr
                cptr += 1
                pair, qb, k = ctx_order[c]
                h0, h1 = 2 * pair, 2 * pair + 1
                if k == 0:
                    ctx_tiles[c // SC] = (
                        ctxps.tile([65, QB], F32, tag="ctx0", name="ctx0"),
                        ctxps.tile([65, QB], F32, tag="ctx1", name="ctx1"),
                    )
                ctx0, ctx1 = ctx_tiles[c // SC]
                et = ets.pop(ctx_order[c])
                nc.tensor.matmul(
                    ctx0[:],
                    v_sb[k][:, h0, :],
                    et[:, 0:QB],
                    start=(k == 0),
                    stop=(k == SC - 1),
                )
                nc.tensor.matmul(
                    ctx1[:],
                    v_sb[k][:, h1, :],
                    et[:, QB : 2 * QB],
                    start=(k == 0),
                    stop=(k == SC - 1),
                )
                if k == SC - 1:
                    # postlude: copy both heads' ctx out of PSUM now (frees
                    # the ctx tiles for the next qb), then queue per-chunk
                    # normalize/transpose/DMA units to drain one per slot so
                    # the postlude never lumps onto a single slot
                    csts = []
                    for h, ctx in ((h0, ctx0), (h1, ctx1)):
                        cts = ctsp.tile([65, QB], BF16, tag="cts", name="cts_t")
                        nc.vector.tensor_copy(cts[:], ctx[:])
                        csts.append((h, cts))

                    def chunk_unit(i, qb=qb, pair=pair, csts=csts):
                        qc = qb * (QB // 128) + i
                        tp = vdps.tile([128, 2, 66], BF16, tag="vd", name="tp_t")
                        for hx, (h, cts) in enumerate(csts):
                            nc.tensor.transpose(
                                tp[:, hx, 0:65],
                                cts[:, i * 128 : (i + 1) * 128],
                                ident_bf[:],
                            )
                        for hx, (h, cts) in enumerate(csts):
                            rc = rcp.tile([128, 1], F32, tag="rc", name="rc_t")
                            nc.vector.reciprocal(rc[:], tp[:, hx, HD : HD + 1])
                            if pair == 1 and qb == NQB - 1:
                                # final qb drains after the last exp - the
                                # ACT engine is idle, so the normalize mul
                                # runs there (Copy with per-partition scale)
                                # in parallel with the DVE recips
                                nc.scalar.activation(
                                    out_sb[qc][:, h * HD : (h + 1) * HD],
                                    tp[:, hx, 0:HD],
                                    mybir.ActivationFunctionType.Copy,
                                    bias=0.0,
                                    scale=rc[:],
                                )
                            else:
                                nc.vector.tensor_scalar_mul(
                                    out_sb[qc][:, h * HD : (h + 1) * HD],
                                    tp[:, hx, 0:HD],
                                    rc[:],
                                )
                        if pair == 1:
                            nc.sync.dma_start(
                                out[qc * 128 : (qc + 1) * 128, :], out_sb[qc][:]
                            )

                    for i in range(QB // 128):
                        postq.append(lambda i=i: chunk_unit(i))

            emit_scores(0)
            for i in range(NSLOT):
                pair, qb, k = slots[i]
                if i + 1 < NSLOT:
                    emit_scores(i + 1)
                st = sts.pop(i)
                et = etp.tile([128, 2 * QB], BF16, tag="et", name="et_t")
                nc.scalar.activation(
                    et[:], st[:], EXP, bias=mask_sb[:, k : k + 1], scale=1.0
                )
                ets[slots[i]] = et
                for fn in fill.get(i, ()):
                    fn()
                for _ in range(ctx_quota[i]):
                    emit_ctx()
                if postq:
                    postq.pop(0)()
            while cptr < NSLOT:
                emit_ctx()
            while postq:
                postq.pop(0)()

    nc.compile()
    return nc


def make_in_maps(hidden_states, attention_mask, Wq, bq, Wk, bk, Wv, bv):
    hidden_states = np.asarray(hidden_states, dtype=np.float32)
    attention_mask = np.asarray(attention_mask, dtype=np.float32)
    Wq = np.asarray(Wq, dtype=np.float32)
    bq = np.asarray(bq, dtype=np.float32)
    Wk = np.asarray(Wk, dtype=np.float32)
    bk = np.asarray(bk, dtype=np.float32)
    Wv = np.asarray(Wv, dtype=np.float32)
    bv = np.asarray(bv, dtype=np.float32)
    bf = ml_dtypes.bfloat16

    def w_layout(WT):
        # [H, DPC] -> [128, JC*DPC] (j-major), used for wv
        return np.ascontiguousarray(
            WT.reshape(JC, 128, DPC).transpose(1, 0, 2).reshape(128, JC * DPC)
        ).astype(bf)

    def w_layout_halves(WT):
        # [H, DPC] -> [128, 2, JC, 128] flattened: each 128-dim p-half is a
        # contiguous 2KB row per partition so its DMA isn't packet-chopped
        return np.ascontiguousarray(
            WT.reshape(JC, 128, 2, 128)
            .transpose(1, 2, 0, 3)
            .reshape(128, JC * DPC)
        ).astype(bf)

    in_maps = []
    for c in range(NCORES):
        b = c // 4
        g = c % 4
        rows = slice(g * DPC, (g + 1) * DPC)
        in_maps.append(
            {
                "hidb": np.ascontiguousarray(
                    hidden_states[b]
                    .T.reshape(JC, 128, 4, 512)
                    .transpose(1, 2, 0, 3)
                    .reshape(128, 4 * JC * 512)
                ).astype(bf),
                "wq": w_layout_halves(Wq[rows, :].T * 0.125),
                "wk": w_layout_halves(Wk[rows, :].T),
                "wv": w_layout(Wv[rows, :].T),
                "bqs": np.ascontiguousarray((bq[rows] * 0.125).reshape(2, 128).T),
                "bks": np.ascontiguousarray(bk[rows].reshape(2, 128).T),
                "bvs": np.ascontiguousarray(bv[rows].reshape(1, DPC)).astype(bf),
                "mask": np.ascontiguousarray(
                    attention_mask[b, 0, 0, :].reshape(SC, 128).T
                ),
            }
        )
    return in_maps


def gather(results):
    full = np.empty((B, S, H), dtype=np.float32)
    for c in range(NCORES):
        b = c // 4
        g = c % 4
        full[b, :, g * DPC : (g + 1) * DPC] = results[c]["out"]
    return full


_NC = None


def kernel(hidden_states, attention_mask, Wq, bq, Wk, bk, Wv, bv, **run_kwargs):
    global _NC
    if _NC is None:
        _NC = build()
    in_maps = make_in_maps(hidden_states, attention_mask, Wq, bq, Wk, bk, Wv, bv)
    res = run_bass_kernel_spmd(_NC, in_maps, core_ids=list(range(NCORES)), **run_kwargs)
    out = gather(res.results)
    if run_kwargs:
        kernel.last_result = res
    return out

